# revision 2
# baseline (speedup 1.0000x reference)
"""Trainium2 Bass kernel for nn_Critic (gnn_message_passing).

Strategy (pure data-parallel over batch, 8 cores x 128 rows):

The reference attention is algebraically collapsed: for single-query
attention, q.(feat@Wk) == feat.(Wk@q), so instead of materializing
[B,N,V] key/value projections we compute a per-row 14-vector
qk[b] = ego'[b] @ (Wq @ Wk^T) and score s[b,n] = feat[b,n,:] . qk[b].
Similarly out = (softmax @ feat) @ Wv, pooling feat first (14 dims).

The subject-bus id subtraction (cols 0/7) shifts scores by a per-row
constant -> cancels in softmax; its effect on the pooled output is a
rank-1 term  -subj_id * sum(w) * (Wv[0]+Wv[7]), appended as an extra
contraction row.  BatchNorm (training mode, global batch stats) needs a
cross-core AllReduce of per-feature sum/sumsq; the BN affine is then
folded into the head-MLP weights (W1' = diag(s)@W1, b1' = b1 + t0@W1),
so no elementwise BN over activations is ever done.

elu(x) is composed as relu(x) + min(exp(x),1) - 1 with the -1 folded
into the scalar output bias via column sums of t_W2.
"""

import numpy as np
from contextlib import ExitStack

import concourse.bacc as bacc
import concourse.tile as tile
from concourse import mybir
import concourse.bass as bass
from concourse.bass_utils import run_bass_kernel_spmd
from concourse.masks import make_identity

B, N, V = 1024, 256, 200
NC = 8
BS = B // NC  # 128 rows per core
F32 = mybir.dt.float32
ALU = mybir.AluOpType
ACTF = mybir.ActivationFunctionType
SCALE = float(1.0 / np.sqrt(V))
NEG = -1.0e9

_cache = {}


def build_nc():
    import os
    STAGE = int(os.environ.get("K_STAGE", "9"))
    nc = bacc.Bacc(None)

    # ---- kernel I/O ----
    mp = nc.dram_tensor("mp", [BS, 15 * N], F32, kind="ExternalInput")  # planes
    egoT = nc.dram_tensor("egoT", [10, BS], F32, kind="ExternalInput")
    wqT = nc.dram_tensor("wqT", [V, 21], F32, kind="ExternalInput")
    wkT = nc.dram_tensor("wkT", [V, 35], F32, kind="ExternalInput")
    wv = nc.dram_tensor("wv", [14, 600], F32, kind="ExternalInput")
    wv07 = nc.dram_tensor("wv07", [1, 1200], F32, kind="ExternalInput")
    w1 = nc.dram_tensor("w1", [V, 600], F32, kind="ExternalInput")
    w2 = nc.dram_tensor("w2", [V, 3], F32, kind="ExternalInput")
    b1T = nc.dram_tensor("b1T", [V, 3], F32, kind="ExternalInput")
    ew1 = nc.dram_tensor("ew1", [4, V], F32, kind="ExternalInput")
    ew2 = nc.dram_tensor("ew2", [V, V], F32, kind="ExternalInput")
    ew3 = nc.dram_tensor("ew3", [V, 1], F32, kind="ExternalInput")
    eb1T = nc.dram_tensor("eb1T", [V, 1], F32, kind="ExternalInput")
    eb2T = nc.dram_tensor("eb2T", [V, 1], F32, kind="ExternalInput")
    gamT = nc.dram_tensor("gamT", [V, 1], F32, kind="ExternalInput")
    betT = nc.dram_tensor("betT", [V, 1], F32, kind="ExternalInput")
    bsum4 = nc.dram_tensor("bsum4", [1, 4], F32, kind="ExternalInput")
    out = nc.dram_tensor("out", [BS, 1], F32, kind="ExternalOutput")

    VC = [(0, 128), (128, 200)]  # v-dim chunks

    with tile.TileContext(nc) as tc:
        with ExitStack() as ctx:
            sb1 = ctx.enter_context(tc.tile_pool(name="sb1", bufs=1))
            ps = ctx.enter_context(tc.tile_pool(name="ps", bufs=2, space="PSUM"))
            ps3 = ctx.enter_context(tc.tile_pool(name="ps3", bufs=3, space="PSUM"))
            psg = ctx.enter_context(tc.tile_pool(name="psg", bufs=1, space="PSUM"))
            dram = ctx.enter_context(tc.tile_pool(name="dram", bufs=1, space="DRAM"))

            # ---------------- DMA in ----------------
            # per-plane tiles for fine-grained overlap
            planes = []
            for f in [2, 14, 0, 1, 3, 4, 5, 6, 7, 8, 9, 10, 11, 12, 13]:
                pl = sb1.tile([BS, N], F32, tag=f"pl{f}", name=f"pl{f}")
                nc.sync.dma_start(out=pl, in_=mp[:, f * N:(f + 1) * N])
                planes.append((f, pl))
            planes = dict(planes)
            loc, flag = planes[2], planes[14]

            ego_sb = sb1.tile([6, BS], F32)
            nc.sync.dma_start(out=ego_sb, in_=egoT[0:6, :])
            egoM_sb = sb1.tile([4, BS], F32)
            nc.sync.dma_start(out=egoM_sb, in_=egoT[6:10, :])
            wqT_sb = [sb1.tile([c1 - c0, 21], F32, tag=f"wq{i}", name=f"wq{i}") for i, (c0, c1) in enumerate(VC)]
            wkT_sb = [sb1.tile([c1 - c0, 35], F32, tag=f"wk{i}", name=f"wk{i}") for i, (c0, c1) in enumerate(VC)]
            for i, (c0, c1) in enumerate(VC):
                nc.sync.dma_start(out=wqT_sb[i], in_=wqT[c0:c1, :])
                nc.sync.dma_start(out=wkT_sb[i], in_=wkT[c0:c1, :])
            wv_sb = sb1.tile([15, 600], F32)
            nc.sync.dma_start(out=wv_sb[0:14, :], in_=wv[:])
            wv07_sb = sb1.tile([1, 1200], F32)
            nc.sync.dma_start(out=wv07_sb, in_=wv07[:])
            w1_sb = [sb1.tile([c1 - c0, 600], F32, tag=f"w1{i}", name=f"w1{i}") for i, (c0, c1) in enumerate(VC)]
            w2_sb = [sb1.tile([c1 - c0, 3], F32, tag=f"w2{i}", name=f"w2{i}") for i, (c0, c1) in enumerate(VC)]
            b1T_sb = [sb1.tile([c1 - c0, 3], F32, tag=f"b1T{i}", name=f"b1T{i}") for i, (c0, c1) in enumerate(VC)]
            ew2_sb = [sb1.tile([c1 - c0, V], F32, tag=f"ew2{i}", name=f"ew2{i}") for i, (c0, c1) in enumerate(VC)]
            ew3_sb = [sb1.tile([c1 - c0, 1], F32, tag=f"ew3{i}", name=f"ew3{i}") for i, (c0, c1) in enumerate(VC)]
            eb1T_sb = [sb1.tile([c1 - c0, 1], F32, tag=f"eb1{i}", name=f"eb1{i}") for i, (c0, c1) in enumerate(VC)]
            eb2T_sb = [sb1.tile([c1 - c0, 1], F32, tag=f"eb2{i}", name=f"eb2{i}") for i, (c0, c1) in enumerate(VC)]
            gamT_sb = [sb1.tile([c1 - c0, 1], F32, tag=f"gam{i}", name=f"gam{i}") for i, (c0, c1) in enumerate(VC)]
            betT_sb = [sb1.tile([c1 - c0, 1], F32, tag=f"bet{i}", name=f"bet{i}") for i, (c0, c1) in enumerate(VC)]
            for i, (c0, c1) in enumerate(VC):
                nc.sync.dma_start(out=w1_sb[i], in_=w1[c0:c1, :])
                nc.sync.dma_start(out=w2_sb[i], in_=w2[c0:c1, :])
                nc.sync.dma_start(out=b1T_sb[i], in_=b1T[c0:c1, :])
                nc.sync.dma_start(out=ew2_sb[i], in_=ew2[c0:c1, :])
                nc.sync.dma_start(out=ew3_sb[i], in_=ew3[c0:c1, :])
                nc.sync.dma_start(out=eb1T_sb[i], in_=eb1T[c0:c1, :])
                nc.sync.dma_start(out=eb2T_sb[i], in_=eb2T[c0:c1, :])
                nc.sync.dma_start(out=gamT_sb[i], in_=gamT[c0:c1, :])
                nc.sync.dma_start(out=betT_sb[i], in_=betT[c0:c1, :])
            ew1_sb = sb1.tile([4, V], F32)
            nc.sync.dma_start(out=ew1_sb, in_=ew1[:])
            bsum4_sb = sb1.tile([1, 4], F32)
            nc.sync.dma_start(out=bsum4_sb, in_=bsum4[:])

            ident = sb1.tile([128, 128], F32)
            make_identity(nc, ident)
            ones_col = sb1.tile([128, 1], F32)
            nc.gpsimd.memset(ones_col, 1.0)
            ones_row = sb1.tile([1, 128], F32)
            nc.gpsimd.memset(ones_row, 1.0)
            eps_col = sb1.tile([128, 1], F32)
            nc.gpsimd.memset(eps_col, 1.0e-5)

            # ---------------- query chain (PE) ----------------
            # Wcomb'[6,35]: rows = ego cols 1..6 of (Wq @ Wk^T) * SCALE
            wc_ps = ps.tile([6, 35], F32, tag="sm", name="wc_ps")
            segcols = [(0, 0, 14), (7, 14, 28), (14, 28, 35)]  # (wq col, wk c0, wk c1)
            for si, (qc, k0, k1) in enumerate(segcols):
                for i in range(2):
                    nc.tensor.matmul(
                        wc_ps[:, k0:k1], wqT_sb[i][:, qc + 1:qc + 7], wkT_sb[i][:, k0:k1],
                        start=(i == 0), stop=(i == 1))
            wc_sb = sb1.tile([6, 35], F32)
            nc.scalar.activation(wc_sb, wc_ps, ACTF.Copy, bias=0.0, scale=SCALE)

            # qk[128,35] = ego'[cols 1..5, a] @ Wcomb'
            qk_ps = ps.tile([BS, 35], F32, tag="sm", name="qk_ps")
            nc.tensor.matmul(qk_ps, ego_sb, wc_sb, start=True, stop=True)
            qk_sb = sb1.tile([BS, 35], F32)
            nc.scalar.activation(qk_sb, qk_ps, ACTF.Copy, bias=0.0, scale=1.0)

            # ---------------- masks (DVE) -> score accumulators ----------------
            subj_loc = loc[:, 0:1]
            geM = sb1.tile([BS, N], F32)
            nc.vector.tensor_scalar(geM, loc, subj_loc, NEG, op0=ALU.is_ge, op1=ALU.mult)
            leM = sb1.tile([BS, N], F32)
            nc.vector.tensor_scalar(leM, loc, subj_loc, NEG, op0=ALU.is_le, op1=ALU.mult)
            nfM = sb1.tile([BS, N], F32)
            nc.vector.tensor_scalar(nfM, flag, 1.0e9, NEG, op0=ALU.mult, op1=ALU.add)
            acc = {}
            acc['u'] = sb1.tile([BS, N], F32, tag="accu", name="accu")
            nc.vector.tensor_tensor(acc['u'], geM, nfM, op=ALU.min)
            acc['d'] = sb1.tile([BS, N], F32, tag="accd", name="accd")
            nc.vector.tensor_tensor(acc['d'], leM, nfM, op=ALU.min)
            acc['p'] = sb1.tile([BS, N], F32, tag="accp", name="accp")
            nc.vector.tensor_scalar(acc['p'], flag, NEG, None, op0=ALU.mult)

            # ---------------- scores (DVE STT) ----------------
            SEG = [('u', 14, 0), ('d', 14, 14), ('p', 7, 28)]
            for s, nf, j0 in SEG:
                for f in range(nf):
                    nc.vector.scalar_tensor_tensor(
                        acc[s], planes[f], qk_sb[:, j0 + f:j0 + f + 1], acc[s],
                        op0=ALU.mult, op1=ALU.add)

            if STAGE <= 1:
                g_sb = sb1.tile([BS, 1], F32, name="g_sb")
                nc.vector.tensor_copy(g_sb, acc['u'][:, 0:1])
                nc.sync.dma_start(out=out[:], in_=g_sb)
                return nc
            # ---------------- softmax exp (ACT) + recip (DVE) ----------------
            w_t, rs_t, wsum1 = {}, {}, {}
            for s, nf, j0 in SEG:
                w_t[s] = sb1.tile([BS, N], F32, tag=f"w{s}", name=f"w{s}")
                se = sb1.tile([BS, 1], F32, tag=f"se{s}", name=f"se{s}")
                nc.scalar.activation(w_t[s], acc[s], ACTF.Exp, bias=0.0, scale=1.0,
                                     accum_out=se)
                seb = sb1.tile([BS, 1], F32, tag=f"seb{s}", name=f"seb{s}")
                nc.vector.tensor_scalar_add(seb, se, 1.0e-30)
                rs_t[s] = sb1.tile([BS, 1], F32, tag=f"rs{s}", name=f"rs{s}")
                nc.vector.reciprocal(rs_t[s], seb)
                wsum1[s] = sb1.tile([BS, 1], F32, tag=f"ws{s}", name=f"ws{s}")
                nc.vector.tensor_tensor(wsum1[s], se, rs_t[s], op=ALU.mult)

            if STAGE <= 2:
                g_sb = sb1.tile([BS, 1], F32, name="g_sb")
                nc.vector.tensor_copy(g_sb, rs_t['u'])
                nc.sync.dma_start(out=out[:], in_=g_sb)
                return nc
            # ---------------- pooled (DVE TTR) ----------------
            scr = sb1.tile([BS, N], F32)  # throwaway elementwise product
            pool = {}
            for s, nf, j0 in SEG:
                pool[s] = sb1.tile([BS, 16], F32, tag=f"pool{s}", name=f"pool{s}")
                for f in range(nf):
                    nc.vector.scalar_tensor_tensor(
                        scr, planes[f], 1.0, w_t[s],
                        op0=ALU.mult, op1=ALU.mult,
                        accum_out=pool[s][:, f:f + 1])
                # normalize + subject row (= subj_id * sum(w_norm))
                nc.vector.tensor_scalar_mul(pool[s][:, 0:nf], pool[s][:, 0:nf], rs_t[s])
                if nf < 14:
                    nc.vector.memset(pool[s][:, nf:14], 0.0)
                nc.vector.tensor_tensor(pool[s][:, 14:15], planes[0][:, 0:1], wsum1[s],
                                        op=ALU.mult)
                nc.vector.memset(pool[s][:, 15:16], 0.0)

            # Wv extension row: -(Wv[0] + Wv[7]) (pv cols of row7 are zero)
            ext_t = sb1.tile([1, 600], F32)
            nc.vector.scalar_tensor_tensor(
                ext_t, wv07_sb[:, 0:600], -1.0, wv07_sb[:, 600:1200],
                op0=ALU.mult, op1=ALU.subtract)
            nc.sync.dma_start(out=wv_sb[14:15, :], in_=ext_t)

            if STAGE <= 3:
                g_sb = sb1.tile([BS, 1], F32, name="g_sb")
                nc.vector.tensor_copy(g_sb, pool['u'][:, 0:1])
                nc.sync.dma_start(out=out[:], in_=g_sb)
                return nc
            # ---------------- pooled^T, u/d/p, stats ----------------
            UU = sb1.tile([BS, 1216], F32)
            nc.vector.memset(UU[:, 1200:1216], 0.0)
            poolT_sb = {}
            xT = {}
            for si, (s, nf, j0) in enumerate(SEG):
                pT = ps.tile([16, BS], F32, tag="sm", name="pT")
                nc.tensor.transpose(pT, pool[s], ident)
                poolT_sb[s] = sb1.tile([16, BS], F32, tag=f"pT{s}", name=f"pT{s}")
                nc.scalar.activation(poolT_sb[s], pT, ACTF.Copy, bias=0.0, scale=1.0)
                # u in [b, v] for stats
                ups = ps3.tile([BS, V], F32, tag="big", name="ups")
                nc.tensor.matmul(ups, poolT_sb[s][0:15, :], wv_sb[:, si * V:(si + 1) * V],
                                 start=True, stop=True)
                nc.scalar.activation(UU[:, si * V:(si + 1) * V], ups, ACTF.Copy,
                                     bias=0.0, scale=1.0)
                # u^T in [v, b] for the head MLP (BN folded into weights later)
                xT[s] = []
                for i, (c0, c1) in enumerate(VC):
                    xps = ps3.tile([c1 - c0, BS], F32, tag="big", name="xps")
                    nc.tensor.matmul(xps, wv_sb[:, si * V + c0:si * V + c1],
                                     poolT_sb[s][0:15, :], start=True, stop=True)
                    xsb = sb1.tile([c1 - c0, BS], F32, tag=f"xT{s}{i}", name=f"xT{s}{i}")
                    nc.scalar.activation(xsb, xps, ACTF.Copy, bias=0.0, scale=1.0)
                    xT[s].append(xsb)

            nc.vector.tensor_tensor(UU[:, 600:1200], UU[:, 0:600], UU[:, 0:600],
                                    op=ALU.mult)

            if STAGE <= 4:
                g_sb = sb1.tile([BS, 1], F32, name="g_sb")
                nc.vector.tensor_copy(g_sb, UU[:, 0:1])
                nc.sync.dma_start(out=out[:], in_=g_sb)
                return nc
            # column sums, transposed: st2[v-part, j] via matmul(UU-cols, ones)
            in_b = dram.tile([V, 6], F32)
            st_sb = []
            for i, (c0, c1) in enumerate(VC):
                pc = c1 - c0
                stp = ps.tile([pc, 6], F32, tag="sm", name=f"stp{i}")
                for j in range(6):
                    nc.tensor.matmul(stp[:, j:j + 1], UU[:, j * 200 + c0:j * 200 + c1],
                                     ones_col, start=True, stop=True)
                t = sb1.tile([pc, 6], F32, tag=f"stsb{i}", name=f"stsb{i}")
                nc.vector.tensor_copy(t, stp)
                st_sb.append(t)
                nc.sync.dma_start(out=in_b[c0:c1, :], in_=t)
            out_b = dram.tile([V, 6], F32, addr_space="Shared")
            if __import__("os").environ.get("NO_CC"):
                nc.sync.dma_start(out=out_b[:], in_=in_b[:])
            else:
                nc.gpsimd.collective_compute(
                    "AllReduce", ALU.add, ins=[in_b[:]], outs=[out_b[:]],
                    replica_groups=[list(range(NC))])

            if STAGE <= 5:
                g_sb = sb1.tile([BS, 1], F32, name="g_sb")
                nc.vector.tensor_copy(g_sb, st_sb[0][:, 0:1])
                nc.sync.dma_start(out=out[:], in_=g_sb)
                return nc
            # ---------------- BN affine from global stats ----------------
            # stat2[v-part, j]: j in {sum_u, sum_d, sum_p, sq_u, sq_d, sq_p}
            s3_t, t03_t = [], []
            for i, (c0, c1) in enumerate(VC):
                pc = c1 - c0
                st = sb1.tile([pc, 6], F32, tag=f"st{i}", name=f"st{i}")
                nc.sync.dma_start(out=st, in_=out_b[c0:c1, :])
                nc.vector.tensor_scalar_mul(st, st, 1.0 / B)  # means
                sq = sb1.tile([pc, 3], F32, tag=f"sq{i}", name=f"sq{i}")
                nc.vector.tensor_tensor(sq, st[:, 0:3], st[:, 0:3], op=ALU.mult)
                var = sb1.tile([pc, 3], F32, tag=f"var{i}", name=f"var{i}")
                nc.vector.tensor_tensor(var, st[:, 3:6], sq, op=ALU.subtract)
                std = sb1.tile([pc, 3], F32, tag=f"std{i}", name=f"std{i}")
                nc.scalar.activation(std, var, ACTF.Sqrt, bias=eps_col[0:pc, :], scale=1.0)
                rstd = sb1.tile([pc, 3], F32, tag=f"rstd{i}", name=f"rstd{i}")
                nc.vector.reciprocal(rstd, std)
                gam_b = bass.AP(tensor=gamT_sb[i].tensor, offset=gamT_sb[i].offset,
                                ap=[gamT_sb[i].ap[0], [0, 3]])
                bet_b = bass.AP(tensor=betT_sb[i].tensor, offset=betT_sb[i].offset,
                                ap=[betT_sb[i].ap[0], [0, 3]])
                s3 = sb1.tile([pc, 3], F32, tag=f"s3{i}", name=f"s3{i}")
                nc.vector.tensor_tensor(s3, rstd, gam_b, op=ALU.mult)
                z3 = sb1.tile([pc, 3], F32, tag=f"z3{i}", name=f"z3{i}")
                nc.vector.tensor_tensor(z3, st[:, 0:3], s3, op=ALU.mult)
                t03 = sb1.tile([pc, 3], F32, tag=f"t03{i}", name=f"t03{i}")
                nc.vector.tensor_tensor(t03, bet_b, z3, op=ALU.subtract)
                s3_t.append(s3)
                t03_t.append(t03)

            # W1' = diag(s) @ W1  (per head, per v-chunk)
            w1p = []
            for i, (c0, c1) in enumerate(VC):
                t = sb1.tile([c1 - c0, 600], F32, tag=f"w1p{i}", name=f"w1p{i}")
                for k in range(3):
                    nc.vector.tensor_scalar_mul(
                        t[:, k * V:(k + 1) * V], w1_sb[i][:, k * V:(k + 1) * V],
                        s3_t[i][:, k:k + 1])
                w1p.append(t)

            # b1' = b1 + t0 @ W1 (raw W1), computed transposed [w,1] per head
            B1 = []
            for j, (w0, w1c) in enumerate(VC):
                pc = w1c - w0
                bt = sb1.tile([pc, 3], F32, tag=f"B1{j}", name=f"B1{j}")
                for k in range(3):
                    bp = ps.tile([pc, 1], F32, tag="sm", name="bp")
                    for i in range(2):
                        nc.tensor.matmul(bp, w1_sb[i][:, k * V + w0:k * V + w1c],
                                         t03_t[i][:, k:k + 1],
                                         start=(i == 0), stop=(i == 1))
                    nc.vector.tensor_copy(bt[:, k:k + 1], bp)
                nc.vector.tensor_tensor(bt, bt, b1T_sb[j], op=ALU.add)
                B1.append(bt)

            if STAGE <= 6:
                g_sb = sb1.tile([BS, 1], F32, name="g_sb")
                nc.vector.tensor_copy(g_sb, w1p[0][:, 0:1])
                nc.sync.dma_start(out=out[:], in_=g_sb)
                return nc
            # ---------------- G accumulation (heads + ego MLP + biases) ----------------
            # ego-MLP (independent of collective): q1 = relu(ego_t@eW1+eb1)
            q1T, q2T = [], []
            for j, (w0, w1c) in enumerate(VC):
                pc = w1c - w0
                qp = ps3.tile([pc, BS], F32, tag="big", name="qp")
                nc.tensor.matmul(qp, ew1_sb[:, w0:w1c], egoM_sb,
                                 start=True, stop=True)
                qs = sb1.tile([pc, BS], F32, tag=f"q1T{j}", name=f"q1T{j}")
                nc.scalar.activation(qs, qp, ACTF.Relu, bias=eb1T_sb[j], scale=1.0)
                q1T.append(qs)
            for j, (w0, w1c) in enumerate(VC):
                pc = w1c - w0
                qp = ps3.tile([pc, BS], F32, tag="big", name="qp2")
                for i in range(2):
                    nc.tensor.matmul(qp, ew2_sb[i][:, w0:w1c], q1T[i],
                                     start=(i == 0), stop=(i == 1))
                qs = sb1.tile([pc, BS], F32, tag=f"q2T{j}", name=f"q2T{j}")
                nc.scalar.activation(qs, qp, ACTF.Relu, bias=eb2T_sb[j], scale=1.0)
                q2T.append(qs)

            G = psg.tile([BS, 1], F32)
            nmm = 0
            # Q1 = q2 @ eW3  (2 matmuls)
            for i in range(2):
                nc.tensor.matmul(G, q2T[i], ew3_sb[i], start=(nmm == 0), stop=False,
                                 skip_group_check=True)
                nmm += 1

            # bias constant: sum(b2)+eb3 - sum_k sum_w W2[w,k]
            wsp = ps.tile([1, 3], F32, tag="sm", name="wsp")
            for i in range(2):
                nc.tensor.matmul(wsp, ones_col[0:VC[i][1] - VC[i][0], :], w2_sb[i],
                                 start=(i == 0), stop=(i == 1))
            wss = sb1.tile([1, 3], F32)
            nc.vector.tensor_copy(wss, wsp)
            r1 = sb1.tile([1, 1], F32)
            nc.vector.reduce_sum(r1, bsum4_sb, axis=mybir.AxisListType.X)
            r2 = sb1.tile([1, 1], F32)
            nc.vector.reduce_sum(r2, wss, axis=mybir.AxisListType.X)
            bs_tot = sb1.tile([1, 1], F32)
            nc.vector.tensor_tensor(bs_tot, r1, r2, op=ALU.subtract)
            nc.tensor.matmul(G, ones_row, bs_tot, start=False, stop=False,
                             skip_group_check=True)
            nmm += 1

            # heads: hT = elu(W1'^T @ xT + b1') + 1 (the +1 folded into bias const)
            for k, s in enumerate(['u', 'd', 'p']):
                for j, (w0, w1c) in enumerate(VC):
                    pc = w1c - w0
                    hp = ps3.tile([pc, BS], F32, tag="big", name="hp")
                    for i in range(2):
                        nc.tensor.matmul(hp, w1p[i][:, k * V + w0:k * V + w1c],
                                         xT[s][i], start=(i == 0), stop=(i == 1))
                    eh = sb1.tile([pc, BS], F32, tag=f"eh{j}", name=f"eh{j}")
                    nc.scalar.activation(eh, hp, ACTF.Exp, bias=B1[j][:, k:k + 1],
                                         scale=1.0)
                    rh = sb1.tile([pc, BS], F32, tag=f"rh{j}", name=f"rh{j}")
                    nc.scalar.activation(rh, hp, ACTF.Relu, bias=B1[j][:, k:k + 1],
                                         scale=1.0)
                    ht = sb1.tile([pc, BS], F32, tag=f"ht{j}", name=f"ht{j}")
                    nc.vector.scalar_tensor_tensor(ht, eh, 1.0, rh,
                                                   op0=ALU.min, op1=ALU.add)
                    nc.tensor.matmul(G, ht, w2_sb[j][:, k:k + 1], start=False,
                                     stop=(k == 2 and j == 1), skip_group_check=True)
                    nmm += 1

            g_sb = sb1.tile([BS, 1], F32)
            nc.vector.tensor_copy(g_sb, G)
            nc.sync.dma_start(out=out[:], in_=g_sb)

    nc.finalize()
    return nc


def prep_inputs(inputs):
    """Host-side layout-only prep (shard, transpose, concat, pad)."""
    merged = np.ascontiguousarray(inputs["merged"], dtype=np.float32)
    a = np.ascontiguousarray(inputs["a"], dtype=np.float32)

    up_Wq, up_Wk, up_Wv = inputs["up_Wq"], inputs["up_Wk"], inputs["up_Wv"]
    dn_Wq, dn_Wk, dn_Wv = inputs["dn_Wq"], inputs["dn_Wk"], inputs["dn_Wv"]
    pv_Wq, pv_Wk, pv_Wv = inputs["pv_Wq"], inputs["pv_Wk"], inputs["pv_Wv"]
    t_W1, t_b1, t_W2, t_b2 = inputs["t_W1"], inputs["t_b1"], inputs["t_W2"], inputs["t_b2"]
    e_W1, e_b1, e_W2, e_b2 = inputs["e_W1"], inputs["e_b1"], inputs["e_W2"], inputs["e_b2"]
    e_W3, e_b3 = inputs["e_W3"], inputs["e_b3"]
    gamma, beta = inputs["gamma"], inputs["beta"]

    f32 = lambda x: np.ascontiguousarray(x, dtype=np.float32)
    wqT = f32(np.concatenate([up_Wq.T, dn_Wq.T, pv_Wq.T], axis=1))        # [200,21]
    wkT = f32(np.concatenate([up_Wk.T, dn_Wk.T, pv_Wk.T], axis=1))        # [200,35]
    pvv = np.zeros((14, V), np.float32)
    pvv[0:7] = pv_Wv
    wv = f32(np.concatenate([up_Wv, dn_Wv, pvv], axis=1))                 # [14,600]
    wv07 = f32(np.concatenate([wv[0], wv[7]]))[None, :]                   # [1,1200]
    w1 = f32(np.concatenate([t_W1[0], t_W1[1], t_W1[2]], axis=1))         # [200,600]
    w2 = f32(t_W2[:, :, 0].T)                                             # [200,3]
    b1T = f32(t_b1.T)                                                     # [200,3]
    ew1 = f32(e_W1)
    ew2 = f32(e_W2)
    ew3 = f32(e_W3)
    eb1T = f32(e_b1[:, None])
    eb2T = f32(e_b2[:, None])
    gamT = f32(gamma[:, None])
    betT = f32(beta[:, None])
    bsum4 = f32(np.concatenate([t_b2[:, 0], e_b3]))[None, :]              # [1,4]

    shared = dict(wqT=wqT, wkT=wkT, wv=wv, wv07=wv07, w1=w1, w2=w2, b1T=b1T, ew1=ew1,
                  ew2=ew2, ew3=ew3, eb1T=eb1T, eb2T=eb2T, gamT=gamT, betT=betT,
                  bsum4=bsum4)

    in_maps = []
    for c in range(NC):
        sh = merged[c * BS:(c + 1) * BS]                                  # [128,256,15]
        mp = f32(sh.transpose(0, 2, 1).reshape(BS, 15 * N))               # planes
        egoT = np.zeros((10, BS), np.float32)
        egoT[0:5] = sh[:, 0, 1:6].T
        egoT[5] = a[c * BS:(c + 1) * BS]
        egoT[6:9] = sh[:, 0, 3:6].T
        egoT[9] = a[c * BS:(c + 1) * BS]
        m = dict(shared)
        m["mp"] = mp
        m["egoT"] = f32(egoT)
        in_maps.append(m)
    return in_maps


def _build():
    nc = build_nc()
    if not nc.is_finalized():
        nc.finalize()
    return nc


def kernel(**inputs):
    if "nc" not in _cache:
        _cache["nc"] = _build()
    nc = _cache["nc"]
    in_maps = prep_inputs(inputs)
    r = run_bass_kernel_spmd(nc, in_maps, list(range(NC)), trace=False)
    out = np.concatenate([r.results[c]["out"] for c in range(NC)], axis=0)
    return out.reshape(-1, 1).astype(np.float32)


def kernel_profiled(inputs, trace_cores=None):
    """Like kernel() but traces execution; returns (out, BassKernelResults)."""
    if "nc" not in _cache:
        _cache["nc"] = _build()
    nc = _cache["nc"]
    in_maps = prep_inputs(inputs)
    r = run_bass_kernel_spmd(nc, in_maps, list(range(NC)), trace=True,
                             trace_cores=trace_cores)
    out = np.concatenate([r.results[c]["out"] for c in range(NC)], axis=0)
    return out.reshape(-1, 1).astype(np.float32), r

